# revision 22
# baseline (speedup 1.0000x reference)
"""Trainium2 Bass kernel for LowDimProjectedAttention.

Model (reference):
  Q = x @ Wq.T + bq ; K,V likewise  (d_model=2048 -> r=512)
  16 heads of d_k=32, softmax(QK^T/sqrt(32)) @ V, then out_proj r->d_model.
  B=2, S=2048. mask is all-ones (verified by spec fill), dropout p=0.

Sharding (8 cores): core c handles batch b=c//4 and heads 4j..4j+4 where
j=c%4 (i.e. 128 of the 512 r-channels, column-parallel QKV). Attention is
fully local per core. A 4-way AllGather inside each batch group rebuilds
attn_out^T, after which each core computes a 512-wide slice of the output
d_model dimension (column-parallel out_proj, bias folded per-partition).

Layouts: all activations live transposed on-chip ([feature, token]); the
host pre-transposes x and the weights (cast to bf16) so no on-device
transpose of x is ever needed. Scores are computed as S^T[k,q] tiles.

Structure per q tile (the part that matters for speed):
  - scores come out in 3-slot PSUM groups, exp'd by ACT into bf16 attn
    tiles. ACT throughput (~0.83ns/elem) is the per-tile floor.
  - the AV matmul's stationary is [V_h | ones] (64 cols), so ONE matmul
    per (kc,h) yields both the weighted V sum and the softmax denominator
    broadcast over 32 partitions -- the separate ones-matmul of the naive
    version is folded away (halves attention-phase PE work).
  - AV matmuls are interleaved between score groups as soon as their attn
    slots are exp'd (one group of delay), so PE and ACT run concurrently
    all tile long.
  - tile q's AV tail + normalize + AllGather launch are emitted at the
    start of tile q+1 (ACT rolls straight across the boundary), and its
    out_proj is emitted mid-tile q+2: the AllGather then has ~2.3 tiles
    of latency budget, so the in-order PE queue doesn't stall on the
    collective even when a peer core runs tens of us late.

All matmuls run at 1 cycle/row (bf16 == float32r rate on TRN2); bf16 is
used for x/W/Q/K/V/attn and the collective to halve DMA + network bytes.
The softmax scale 1/sqrt(32) is folded into Wq/bq on the host.
"""

import math

import numpy as np

B = 2
S = 2048
D_MODEL = 2048
R = 512
N_HEADS = 16
D_K = 32
N_CORES = 8
GROUP = 4          # cores per batch group
RLOC = 128         # r-channels per core (4 heads x 32)
NH = 4             # heads per core
TQ = 512           # q tile size
NQT = S // TQ      # 4 q tiles
NKT = S // 128     # 16 k chunks
NDM = D_MODEL // 128  # 16 d_model chunks
SLOT_GROUP = 3     # score slots per ACT exp instruction (3 psum banks)
N_SLOTS = NKT * NH  # 64 score tiles per q tile: slot = 4*kc + h

_CACHE = {}
TRACE = False
LAST_RESULT = None


def _build():
    import concourse.mybir as mybir
    import concourse.tile as tile
    from concourse import bacc
    from concourse.masks import make_identity

    F32 = mybir.dt.float32
    BF16 = mybir.dt.bfloat16

    # Bacc (not plain Bass): its finalize() runs move_matmul_waits_to_
    # ldweights / generate_event_semaphores etc., without which walrus
    # rejects multi-wait instructions ("Too many sync wait commands").
    nc = bacc.Bacc("TRN2", target_bir_lowering=False, num_devices=N_CORES)

    xT = nc.dram_tensor("xT", [D_MODEL, S], BF16, kind="ExternalInput")
    wqT = nc.dram_tensor("wqT", [D_MODEL, RLOC], BF16, kind="ExternalInput")
    wkT = nc.dram_tensor("wkT", [D_MODEL, RLOC], BF16, kind="ExternalInput")
    wvT = nc.dram_tensor("wvT", [D_MODEL, RLOC], BF16, kind="ExternalInput")
    woTs = nc.dram_tensor("woTs", [R, 512], BF16, kind="ExternalInput")
    bq = nc.dram_tensor("bq", [RLOC, 1], F32, kind="ExternalInput")
    bk = nc.dram_tensor("bk", [RLOC, 1], F32, kind="ExternalInput")
    bv = nc.dram_tensor("bv", [RLOC, 1], F32, kind="ExternalInput")
    bo2 = nc.dram_tensor("bo2", [128, 4], F32, kind="ExternalInput")
    outT = nc.dram_tensor("outT", [512, S], F32, kind="ExternalOutput")

    # Per-q-tile collective bounce buffers (chunked AllGather overlaps the
    # epilogue with attention of later q tiles).
    cc_in = [
        nc.dram_tensor(f"cc_in{i}", [RLOC, TQ], BF16, kind="Internal")
        for i in range(NQT)
    ]
    # NOTE: Shared-output collectives need >4-core groups; Local output is
    # the supported path for 4-core batch groups (extra HBM copy, fine).
    cc_out = [
        nc.dram_tensor(f"cc_out{i}", [R, TQ], BF16, kind="Internal")
        for i in range(NQT)
    ]
    replica_groups = [[0, 1, 2, 3], [4, 5, 6, 7]]

    with tile.TileContext(nc) as tc:
        with (
            tc.tile_pool(name="const", bufs=1) as const,
            tc.tile_pool(name="wpool", bufs=1) as wpool,
            tc.tile_pool(name="xpool", bufs=4) as xpool,
            tc.tile_pool(name="qkv", bufs=1) as qkv,
            tc.tile_pool(name="attnp", bufs=10) as attnp,
            tc.tile_pool(name="denp", bufs=2) as denp,
            tc.tile_pool(name="otp", bufs=2) as otp,
            tc.tile_pool(name="agp", bufs=8) as agp,
            tc.tile_pool(name="outp", bufs=2) as outp,
        ):
            # ---- constants / weights -------------------------------------
            # chunked weight loads: one DMA per 128x128 chunk so each matmul
            # waits on a single DMA-queue semaphore (a single sprayed DMA
            # fans across queues and overflows the ISA wait slots).
            wq_sb = wpool.tile([128, NDM, RLOC], BF16)
            wk_sb = wpool.tile([128, NDM, RLOC], BF16)
            wv_sb = wpool.tile([128, NDM, RLOC], BF16)
            for dm in range(NDM):
                rs = slice(128 * dm, 128 * (dm + 1))
                nc.sync.dma_start(wq_sb[:, dm, :], wqT[rs, :])
                nc.sync.dma_start(wk_sb[:, dm, :], wkT[rs, :])
                nc.sync.dma_start(wv_sb[:, dm, :], wvT[rs, :])
            wo_sb = wpool.tile([128, 4, 4, 128], BF16)
            for rc in range(4):
                for dmt in range(4):
                    nc.sync.dma_start(
                        wo_sb[:, rc, dmt, :],
                        woTs[128 * rc : 128 * (rc + 1), 128 * dmt : 128 * (dmt + 1)],
                    )
            bq_sb = const.tile([RLOC, 1], F32)
            bk_sb = const.tile([RLOC, 1], F32)
            bv_sb = const.tile([RLOC, 1], F32)
            bo_sb = const.tile([128, 4], F32)
            nc.sync.dma_start(bq_sb, bq[:])
            nc.sync.dma_start(bk_sb, bk[:])
            nc.sync.dma_start(bv_sb, bv[:])
            nc.sync.dma_start(bo_sb, bo2[:])

            ident = const.tile([128, 128], BF16)
            make_identity(nc, ident[:])

            # Merged AV stationary: per (kc, h) a [128, 64] block whose
            # first 32 cols are V_h's chunk and last 32 cols are ones, so a
            # single matmul yields [AV_h ; denominator broadcast x32].
            v_mg = qkv.tile([128, NKT, NH, 64], BF16)
            nc.vector.memset(v_mg[:, :, :, 32:64], 1.0)

            qt = qkv.tile([RLOC, S], BF16)
            kt = qkv.tile([RLOC, S], BF16)
            vt_bf = qkv.tile([RLOC, S], BF16)

            # psum budget: 2x3-bank score groups (ping-pong under the exp
            # drain) + a 2-bank AV/den accumulator = exactly 8 banks.
            # QKV projection, V transposes and out_proj all borrow
            # score-pool banks (tag "sc") so both pools span the kernel.
            ps_sc_ctx = tc.tile_pool(name="ps_sc", bufs=2, space="PSUM")
            ps_av_ctx = tc.tile_pool(name="ps_av", bufs=1, space="PSUM")
            ps_sc = ps_sc_ctx.__enter__()
            ps_av = ps_av_ctx.__enter__()

            def emit_epilogue(qe):
                # out_proj for q tile qe (column-parallel over d_model).
                # Called one tile late so the AllGather is already done.
                qsl = slice(TQ * qe, TQ * (qe + 1))
                # gathers go on the Sync queue: on GpSimd they'd sit behind
                # the NEXT tile's collective_compute, which blocks that
                # queue until the AllGather completes -- a late peer then
                # stalls the in-order PE at the epilogue matmuls.
                ag_t = []
                for rc in range(GROUP):
                    t_ = agp.tile([128, TQ], BF16)
                    nc.sync.dma_start(
                        t_, cc_out[qe][128 * rc : 128 * (rc + 1), :]
                    )
                    ag_t.append(t_)
                for dmt in range(4):
                    pso2 = ps_sc.tile([128, TQ], F32, tag="sc")
                    for rc in range(GROUP):
                        nc.tensor.matmul(
                            pso2[:],
                            wo_sb[:, rc, dmt, :],
                            ag_t[rc][:],
                            start=(rc == 0), stop=(rc == GROUP - 1),
                            skip_group_check=True,
                        )
                    ob = outp.tile([128, TQ], F32)
                    nc.vector.tensor_scalar_add(
                        ob[:], pso2[:], bo_sb[:, dmt : dmt + 1]
                    )
                    nc.sync.dma_start(outT[128 * dmt : 128 * (dmt + 1), qsl], ob[:])

            # ---- attention: scores/exp/AV interleaved per q tile ---------
            def make_av_emitter(av, slot_ap):
                def emit_av(kc):
                    for h in range(NH):
                        p0 = 64 * (h % 2)
                        nc.tensor.matmul(
                            av[p0 : p0 + 64, h // 2, :],
                            v_mg[:, kc, h, :],
                            slot_ap[4 * kc + h],
                            start=(kc == 0), stop=(kc == NKT - 1),
                            tile_position=(0, p0),
                            skip_group_check=True,
                        )
                return emit_av

            def flush_and_normalize(pend):
                # Remaining AV chunks, then out = AV / denom, then ship.
                # (The denominator is already broadcast over each head's 32
                # rows by the ones columns of the merged stationary.)
                qf, av, emit_av, next_kc = pend
                for kc in range(next_kc, NKT):
                    emit_av(kc)
                # Stage the accumulator to SBUF with heads realigned to
                # partition band 32h (copies have a single SBUF operand, so
                # the verifier's same-start-partition rule for TensorTensor
                # doesn't bite). The psum banks are released for the next
                # tile's accumulation after these 8 copies (~4us), while the
                # slow exact reciprocal (DVE divide is ~8 cycles/elem; the
                # approx-fast custom op miscompiles under this runner) runs
                # off the critical path. Realignment also lets recip and the
                # AV*recip product each be ONE full-width instruction.
                av_al = denp.tile([128, TQ], F32, tag="aval")
                rbsrc = denp.tile([128, TQ], F32, tag="rbs")
                for h in range(NH):
                    p0 = 64 * (h % 2)
                    nc.vector.tensor_copy(
                        av_al[32 * h : 32 * (h + 1), :],
                        av[p0 : p0 + 32, h // 2, :],
                    )
                for h in range(NH):
                    p0 = 64 * (h % 2)
                    nc.vector.tensor_copy(
                        rbsrc[32 * h : 32 * (h + 1), :],
                        av[p0 + 32 : p0 + 64, h // 2, :],
                    )
                rb = denp.tile([128, TQ], F32, tag="rb")
                nc.vector.reciprocal(rb[:], rbsrc[:])
                ot = otp.tile([128, TQ], BF16)
                nc.vector.tensor_mul(ot[:], av_al[:], rb[:])
                nc.sync.dma_start(cc_in[qf][:], ot[:])
                # gather the 4 cores' head-slices of this q tile
                nc.gpsimd.collective_compute(
                    "AllGather",
                    mybir.AluOpType.bypass,
                    replica_groups=replica_groups,
                    ins=[cc_in[qf][:]],
                    outs=[cc_out[qf][:]],
                )

            # ---- q tile 0, emitted progressively inside the proj loop ----
            # After proj tile t lands (K/Q/V for tokens 512t..512t+512 and
            # the V chunks transposed into v_mg), score slots up to 16(t+1)
            # become legal: emitting them here keeps ACT busy from ~12us in
            # instead of idling through the whole projection phase.
            q0_state = {"g0": 0, "gi": 0, "next_kc": 0, "av": None,
                        "emit_av": None, "slot_ap": {}}

            def emit_q0_groups(limit):
                st = q0_state
                while st["g0"] < limit:
                    g0 = st["g0"]
                    n = min(SLOT_GROUP, limit - g0)
                    pss = ps_sc.tile([128, n * TQ], F32, tag="sc")
                    for s in range(n):
                        kc, h = divmod(g0 + s, NH)
                        nc.tensor.matmul(
                            pss[:, TQ * s : TQ * (s + 1)],
                            kt[32 * h : 32 * (h + 1), 128 * kc : 128 * (kc + 1)],
                            qt[32 * h : 32 * (h + 1), 0:TQ],
                            start=True, stop=True,
                            tile_position=(32 * h, 0),
                            skip_group_check=True,
                        )
                    while st["next_kc"] < NKT and 4 * st["next_kc"] + 4 <= g0:
                        if st["av"] is None:
                            st["av"] = ps_av.tile([128, 2, TQ], F32, name="av")
                            st["emit_av"] = make_av_emitter(
                                st["av"], st["slot_ap"]
                            )
                        st["emit_av"](st["next_kc"])
                        st["next_kc"] += 1
                    att = attnp.tile([128, n * TQ], BF16, tag="at")
                    nc.scalar.activation(
                        att[:], pss[:], mybir.ActivationFunctionType.Exp
                    )
                    for s in range(n):
                        st["slot_ap"][g0 + s] = att[:, TQ * s : TQ * (s + 1)]
                    st["g0"] = g0 + n
                    st["gi"] += 1

            # ---- QKV projections, one pass over x^T, fused with q tile 0 -
            for t in range(NQT):
                tsl = slice(TQ * t, TQ * (t + 1))
                pst3 = ps_sc.tile([128, 3 * TQ], F32, tag="sc")
                psq = pst3[:, 0:TQ]
                psk = pst3[:, TQ : 2 * TQ]
                psv = pst3[:, 2 * TQ : 3 * TQ]
                for dm in range(NDM):
                    xt_t = xpool.tile([128, TQ], BF16)
                    nc.gpsimd.dma_start(xt_t, xT[128 * dm : 128 * (dm + 1), tsl])
                    xr = xt_t[:]
                    nc.tensor.matmul(
                        psq, wq_sb[:, dm, :], xr,
                        start=(dm == 0), stop=(dm == NDM - 1),
                        skip_group_check=True,
                    )
                    nc.tensor.matmul(
                        psk, wk_sb[:, dm, :], xr,
                        start=(dm == 0), stop=(dm == NDM - 1),
                        skip_group_check=True,
                    )
                    nc.tensor.matmul(
                        psv, wv_sb[:, dm, :], xr,
                        start=(dm == 0), stop=(dm == NDM - 1),
                        skip_group_check=True,
                    )
                nc.vector.tensor_scalar_add(qt[:, tsl], psq, bq_sb[:])
                nc.vector.tensor_scalar_add(kt[:, tsl], psk, bk_sb[:])
                nc.vector.tensor_scalar_add(vt_bf[:, tsl], psv, bv_sb[:])
                # this tile's 4 V chunks -> natural layout in the merged
                # [V|ones] stationary (single strided copy for all 4)
                tpt = ps_sc.tile([128, 4, 128], BF16, tag="sc")
                for c4 in range(4):
                    c = 4 * t + c4
                    nc.tensor.matmul(
                        tpt[:, c4, :], vt_bf[:, 128 * c : 128 * (c + 1)],
                        ident[:], is_transpose=True,
                        skip_group_check=True,
                    )
                nc.vector.tensor_copy(v_mg[:, 4 * t : 4 * t + 4, :, 0:32], tpt[:])

            # q tile 0 after the full projection: interleaving its score
            # groups INTO the proj loop measured slower -- the shared "sc"
            # rotation makes each proj tile's psum allocation wait on a
            # score group's exp, coupling the PE to ACT pacing.
            emit_q0_groups(N_SLOTS)

            pending = (
                0, q0_state["av"], q0_state["emit_av"], q0_state["next_kc"]
            )
            for q in range(1, NQT):
                qsl = slice(TQ * q, TQ * (q + 1))

                # AV/den accumulator: heads 0,1 in bank 0 (partitions 0-63 /
                # 64-127), heads 2,3 in bank 1. Within a head's 64
                # partitions: 0-31 = AV, 32-63 = denominator (x32 rows).
                # Allocated lazily at first use: it must be claimed AFTER
                # the previous tile's flush is emitted, or the pool rotation
                # mis-attributes the flush's reads to the new generation and
                # the next tile's accumulation races the normalize.
                av = None
                slot_ap = {}
                emit_av = None
                next_kc = 0
                g0 = 0
                gi = 0
                while g0 < N_SLOTS:
                    n = min(SLOT_GROUP, N_SLOTS - g0)
                    pss = ps_sc.tile([128, n * TQ], F32, tag="sc")
                    for s in range(n):
                        kc, h = divmod(g0 + s, NH)
                        nc.tensor.matmul(
                            pss[:, TQ * s : TQ * (s + 1)],
                            kt[32 * h : 32 * (h + 1), 128 * kc : 128 * (kc + 1)],
                            qt[32 * h : 32 * (h + 1), qsl],
                            start=True, stop=True,
                            tile_position=(32 * h, 0),
                            skip_group_check=True,
                        )
                    # Previous tile's AV tail + normalize + AllGather happen
                    # here, after this tile's first score group: ACT rolls
                    # straight from the old tile's exps into the new one's.
                    # The epilogue runs at pipeline depth 2 (its AllGather
                    # launched a full tile ago), emitted back-to-back after
                    # the flush so its psum accumulation never interleaves
                    # with this tile's AV groups.
                    if gi == 0 and pending is not None:
                        flush_and_normalize(pending)
                        pending = None
                    # AV for every k chunk whose slots were exp'd at least
                    # one group ago (so the in-order PE never waits on the
                    # exp that was issued right before it); ACT chews the
                    # score group just produced in parallel.
                    while next_kc < NKT and 4 * next_kc + 4 <= g0:
                        if av is None:
                            av = ps_av.tile([128, 2, TQ], F32)
                            emit_av = make_av_emitter(av, slot_ap)
                        emit_av(next_kc)
                        next_kc += 1
                    att = attnp.tile([128, n * TQ], BF16, tag="at")
                    nc.scalar.activation(
                        att[:], pss[:], mybir.ActivationFunctionType.Exp
                    )
                    for s in range(n):
                        slot_ap[g0 + s] = att[:, TQ * s : TQ * (s + 1)]
                    g0 += n
                    gi += 1

                    # Epilogue at pipeline depth 2, mid-tile: its AllGather
                    # launched ~2.3 tiles ago, so even a badly skewed peer
                    # core can't stall the in-order PE here.
                    if q >= 2 and gi == 10:
                        emit_epilogue(q - 2)
                pending = (q, av, emit_av, next_kc)

            emit_epilogue(NQT - 2)
            flush_and_normalize(pending)
            emit_epilogue(NQT - 1)

            ps_av_ctx.__exit__(None, None, None)
            ps_sc_ctx.__exit__(None, None, None)

    nc.finalize()
    return nc


def _prepare_inputs(x, Wq, bq, Wk, bk, Wv, bv, Wo, bo):
    import ml_dtypes

    bf16 = ml_dtypes.bfloat16
    scale = 1.0 / math.sqrt(D_K)
    x = np.asarray(x, np.float32)
    in_maps = []
    for c in range(N_CORES):
        b, j = divmod(c, GROUP)
        rsl = slice(RLOC * j, RLOC * (j + 1))
        dsl = slice(512 * j, 512 * (j + 1))
        in_maps.append(
            {
                "xT": np.ascontiguousarray(x[b].T).astype(bf16),
                "wqT": np.ascontiguousarray((np.asarray(Wq)[rsl] * scale).T.astype(np.float32)).astype(bf16),
                "wkT": np.ascontiguousarray(np.asarray(Wk)[rsl].T.astype(np.float32)).astype(bf16),
                "wvT": np.ascontiguousarray(np.asarray(Wv)[rsl].T.astype(np.float32)).astype(bf16),
                "woTs": np.ascontiguousarray(np.asarray(Wo)[dsl].T.astype(np.float32)).astype(bf16),
                "bq": (np.asarray(bq)[rsl] * scale).astype(np.float32).reshape(RLOC, 1),
                "bk": np.asarray(bk)[rsl].astype(np.float32).reshape(RLOC, 1),
                "bv": np.asarray(bv)[rsl].astype(np.float32).reshape(RLOC, 1),
                "bo2": np.ascontiguousarray(
                    np.asarray(bo)[dsl].astype(np.float32).reshape(4, 128).T
                ),
            }
        )
    return in_maps


def kernel(x, Wq, bq, Wk, bk, Wv, bv, Wo, bo, mask=None):
    global LAST_RESULT
    from concourse.bass_utils import run_bass_kernel_spmd

    if "nc" not in _CACHE:
        _CACHE["nc"] = _build()
    nc = _CACHE["nc"]

    in_maps = _prepare_inputs(x, Wq, bq, Wk, bk, Wv, bv, Wo, bo)
    res = run_bass_kernel_spmd(
        nc, in_maps, core_ids=list(range(N_CORES)), trace=TRACE
    )
    LAST_RESULT = res
    out = np.empty((B, S, D_MODEL), np.float32)
    for c in range(N_CORES):
        b, j = divmod(c, GROUP)
        out[b, :, 512 * j : 512 * (j + 1)] = res.results[c]["outT"].T
    return out


# revision 24
# speedup vs baseline: 1.2296x; 1.2296x over previous
"""Trainium2 Bass kernel for LowDimProjectedAttention.

Model (reference):
  Q = x @ Wq.T + bq ; K,V likewise  (d_model=2048 -> r=512)
  16 heads of d_k=32, softmax(QK^T/sqrt(32)) @ V, then out_proj r->d_model.
  B=2, S=2048. mask is all-ones (verified by spec fill), dropout p=0.

Sharding (8 cores): core c handles batch b=c//4 and heads 4j..4j+4 where
j=c%4 (i.e. 128 of the 512 r-channels, column-parallel QKV). Attention is
fully local per core. A 4-way AllGather inside each batch group rebuilds
attn_out^T, after which each core computes a 512-wide slice of the output
d_model dimension (column-parallel out_proj, bias folded per-partition).

Layouts: all activations live transposed on-chip ([feature, token]); the
host pre-transposes x and the weights (cast to bf16) so no on-device
transpose of x is ever needed. Scores are computed as S^T[k,q] tiles.

Structure per q tile (the part that matters for speed):
  - scores come out in 3-slot PSUM groups, exp'd by ACT into bf16 attn
    tiles. ACT throughput (~0.83ns/elem) is the per-tile floor.
  - the AV matmul's stationary is [V_h | ones] (64 cols), so ONE matmul
    per (kc,h) yields both the weighted V sum and the softmax denominator
    broadcast over 32 partitions -- the separate ones-matmul of the naive
    version is folded away (halves attention-phase PE work).
  - AV matmuls are interleaved between score groups as soon as their attn
    slots are exp'd (one group of delay), so PE and ACT run concurrently
    all tile long.
  - tile q's AV tail + normalize + AllGather launch are emitted at the
    start of tile q+1 (ACT rolls straight across the boundary), and its
    out_proj is emitted mid-tile q+2: the AllGather then has ~2.3 tiles
    of latency budget, so the in-order PE queue doesn't stall on the
    collective even when a peer core runs tens of us late.

All matmuls run at 1 cycle/row (bf16 == float32r rate on TRN2); bf16 is
used for x/W/Q/K/V/attn and the collective to halve DMA + network bytes.
The softmax scale 1/sqrt(32) is folded into Wq/bq on the host.
"""

import math

import numpy as np

B = 2
S = 2048
D_MODEL = 2048
R = 512
N_HEADS = 16
D_K = 32
N_CORES = 8
GROUP = 4          # cores per batch group
RLOC = 128         # r-channels per core (4 heads x 32)
NH = 4             # heads per core
TQ = 512           # q tile size
NQT = S // TQ      # 4 q tiles
NKT = S // 128     # 16 k chunks
NDM = D_MODEL // 128  # 16 d_model chunks
SLOT_GROUP = 3     # score slots per ACT exp instruction (3 psum banks)
N_SLOTS = NKT * NH  # 64 score tiles per q tile: slot = 4*kc + h

_CACHE = {}
TRACE = False
LAST_RESULT = None


def _build():
    import concourse.mybir as mybir
    import concourse.tile as tile
    from concourse import bacc
    from concourse.masks import make_identity

    F32 = mybir.dt.float32
    BF16 = mybir.dt.bfloat16

    # Bacc (not plain Bass): its finalize() runs move_matmul_waits_to_
    # ldweights / generate_event_semaphores etc., without which walrus
    # rejects multi-wait instructions ("Too many sync wait commands").
    nc = bacc.Bacc("TRN2", target_bir_lowering=False, num_devices=N_CORES)

    xT = nc.dram_tensor("xT", [D_MODEL, S], BF16, kind="ExternalInput")
    wqT = nc.dram_tensor("wqT", [D_MODEL, RLOC], BF16, kind="ExternalInput")
    wkT = nc.dram_tensor("wkT", [D_MODEL, RLOC], BF16, kind="ExternalInput")
    wvT = nc.dram_tensor("wvT", [D_MODEL, RLOC], BF16, kind="ExternalInput")
    woTs = nc.dram_tensor("woTs", [R, 512], BF16, kind="ExternalInput")
    bq = nc.dram_tensor("bq", [RLOC, 1], F32, kind="ExternalInput")
    bk = nc.dram_tensor("bk", [RLOC, 1], F32, kind="ExternalInput")
    bv = nc.dram_tensor("bv", [RLOC, 1], F32, kind="ExternalInput")
    bo2 = nc.dram_tensor("bo2", [128, 4], F32, kind="ExternalInput")
    outT = nc.dram_tensor("outT", [512, S], F32, kind="ExternalOutput")

    # Per-q-tile collective bounce buffers (chunked AllGather overlaps the
    # epilogue with attention of later q tiles).
    cc_in = [
        nc.dram_tensor(f"cc_in{i}", [RLOC, TQ], BF16, kind="Internal")
        for i in range(NQT)
    ]
    # NOTE: Shared-output collectives need >4-core groups; Local output is
    # the supported path for 4-core batch groups (extra HBM copy, fine).
    cc_out = [
        nc.dram_tensor(f"cc_out{i}", [R, TQ], BF16, kind="Internal")
        for i in range(NQT)
    ]
    replica_groups = [[0, 1, 2, 3], [4, 5, 6, 7]]

    with tile.TileContext(nc) as tc:
        with (
            tc.tile_pool(name="const", bufs=1) as const,
            tc.tile_pool(name="wpool", bufs=1) as wpool,
            tc.tile_pool(name="xpool", bufs=4) as xpool,
            tc.tile_pool(name="qkv", bufs=1) as qkv,
            tc.tile_pool(name="attnp", bufs=12) as attnp,
            tc.tile_pool(name="denp", bufs=2) as denp,
            tc.tile_pool(name="otp", bufs=2) as otp,
            tc.tile_pool(name="agp", bufs=8) as agp,
            tc.tile_pool(name="outp", bufs=2) as outp,
        ):
            # ---- constants / weights -------------------------------------
            # chunked weight loads: one DMA per 128x128 chunk so each matmul
            # waits on a single DMA-queue semaphore (a single sprayed DMA
            # fans across queues and overflows the ISA wait slots).
            wq_sb = wpool.tile([128, NDM, RLOC], BF16)
            wk_sb = wpool.tile([128, NDM, RLOC], BF16)
            wv_sb = wpool.tile([128, NDM, RLOC], BF16)
            for dm in range(NDM):
                rs = slice(128 * dm, 128 * (dm + 1))
                nc.sync.dma_start(wq_sb[:, dm, :], wqT[rs, :])
                nc.sync.dma_start(wk_sb[:, dm, :], wkT[rs, :])
                nc.sync.dma_start(wv_sb[:, dm, :], wvT[rs, :])
            wo_sb = wpool.tile([128, 4, 4, 128], BF16)
            for rc in range(4):
                for dmt in range(4):
                    nc.sync.dma_start(
                        wo_sb[:, rc, dmt, :],
                        woTs[128 * rc : 128 * (rc + 1), 128 * dmt : 128 * (dmt + 1)],
                    )
            bq_sb = const.tile([RLOC, 1], F32)
            bk_sb = const.tile([RLOC, 1], F32)
            bv_sb = const.tile([RLOC, 1], F32)
            bo_sb = const.tile([128, 4], F32)
            nc.sync.dma_start(bq_sb, bq[:])
            nc.sync.dma_start(bk_sb, bk[:])
            nc.sync.dma_start(bv_sb, bv[:])
            nc.sync.dma_start(bo_sb, bo2[:])

            ident = const.tile([128, 128], BF16)
            make_identity(nc, ident[:])

            # Merged AV stationary: per (kc, h) a [128, 64] block whose
            # first 32 cols are V_h's chunk and last 32 cols are ones, so a
            # single matmul yields [AV_h ; denominator broadcast x32].
            v_mg = qkv.tile([128, NKT, NH, 64], BF16)
            nc.vector.memset(v_mg[:, :, :, 32:64], 1.0)

            qt = qkv.tile([RLOC, S], BF16)
            kt = qkv.tile([RLOC, S], BF16)
            vt_bf = qkv.tile([RLOC, S], BF16)

            # psum budget: 2x3-bank score groups (ping-pong under the exp
            # drain) + a 2-bank AV/den accumulator = exactly 8 banks.
            # QKV projection, V transposes and out_proj all borrow
            # score-pool banks (tag "sc") so both pools span the kernel.
            ps_sc_ctx = tc.tile_pool(name="ps_sc", bufs=2, space="PSUM")
            ps_av_ctx = tc.tile_pool(name="ps_av", bufs=1, space="PSUM")
            ps_sc = ps_sc_ctx.__enter__()
            ps_av = ps_av_ctx.__enter__()

            def emit_epilogue(qe):
                # out_proj for q tile qe (column-parallel over d_model).
                # Called one tile late so the AllGather is already done.
                qsl = slice(TQ * qe, TQ * (qe + 1))
                # gathers go on the Sync queue: on GpSimd they'd sit behind
                # the NEXT tile's collective_compute, which blocks that
                # queue until the AllGather completes -- a late peer then
                # stalls the in-order PE at the epilogue matmuls.
                ag_t = []
                for rc in range(GROUP):
                    t_ = agp.tile([128, TQ], BF16)
                    nc.sync.dma_start(
                        t_, cc_out[qe][128 * rc : 128 * (rc + 1), :]
                    )
                    ag_t.append(t_)
                for dmt in range(4):
                    pso2 = ps_sc.tile([128, TQ], F32, tag="sc")
                    for rc in range(GROUP):
                        nc.tensor.matmul(
                            pso2[:],
                            wo_sb[:, rc, dmt, :],
                            ag_t[rc][:],
                            start=(rc == 0), stop=(rc == GROUP - 1),
                            skip_group_check=True,
                        )
                    ob = outp.tile([128, TQ], F32)
                    nc.vector.tensor_scalar_add(
                        ob[:], pso2[:], bo_sb[:, dmt : dmt + 1]
                    )
                    nc.sync.dma_start(outT[128 * dmt : 128 * (dmt + 1), qsl], ob[:])

            # ---- attention: scores/exp/AV interleaved per q tile ---------
            def make_av_emitter(av, slot_ap):
                def emit_av(kc):
                    for h in range(NH):
                        p0 = 64 * (h % 2)
                        nc.tensor.matmul(
                            av[p0 : p0 + 64, h // 2, :],
                            v_mg[:, kc, h, :],
                            slot_ap[4 * kc + h],
                            start=(kc == 0), stop=(kc == NKT - 1),
                            tile_position=(0, p0),
                            skip_group_check=True,
                        )
                return emit_av

            def flush_and_normalize(pend):
                # Remaining AV chunks, then out = AV / denom, then ship.
                # (The denominator is already broadcast over each head's 32
                # rows by the ones columns of the merged stationary.)
                qf, av, emit_av, next_kc = pend
                for kc in range(next_kc, NKT):
                    emit_av(kc)
                # Stage the accumulator to SBUF with heads realigned to
                # partition band 32h (copies have a single SBUF operand, so
                # the verifier's same-start-partition rule for TensorTensor
                # doesn't bite). The psum banks are released for the next
                # tile's accumulation after these 8 copies (~4us), while the
                # slow exact reciprocal (DVE divide is ~8 cycles/elem; the
                # approx-fast custom op miscompiles under this runner) runs
                # off the critical path. Realignment also lets recip and the
                # AV*recip product each be ONE full-width instruction.
                av_al = denp.tile([128, TQ], F32, tag="aval")
                rbsrc = denp.tile([128, TQ], F32, tag="rbs")
                for h in range(NH):
                    p0 = 64 * (h % 2)
                    nc.vector.tensor_copy(
                        av_al[32 * h : 32 * (h + 1), :],
                        av[p0 : p0 + 32, h // 2, :],
                    )
                for h in range(NH):
                    p0 = 64 * (h % 2)
                    nc.vector.tensor_copy(
                        rbsrc[32 * h : 32 * (h + 1), :],
                        av[p0 + 32 : p0 + 64, h // 2, :],
                    )
                rb = denp.tile([128, TQ], F32, tag="rb")
                nc.vector.reciprocal(rb[:], rbsrc[:])
                ot = otp.tile([128, TQ], BF16)
                nc.vector.tensor_mul(ot[:], av_al[:], rb[:])
                nc.sync.dma_start(cc_in[qf][:], ot[:])
                # gather the 4 cores' head-slices of this q tile
                nc.gpsimd.collective_compute(
                    "AllGather",
                    mybir.AluOpType.bypass,
                    replica_groups=replica_groups,
                    ins=[cc_in[qf][:]],
                    outs=[cc_out[qf][:]],
                )

            # ---- q tile 0, emitted progressively inside the proj loop ----
            # After proj tile t lands (K/Q/V for tokens 512t..512t+512 and
            # the V chunks transposed into v_mg), score slots up to 16(t+1)
            # become legal: emitting them here keeps ACT busy from ~12us in
            # instead of idling through the whole projection phase.
            q0_state = {"g0": 0, "gi": 0, "next_kc": 0, "av": None,
                        "emit_av": None, "slot_ap": {}}

            def emit_q0_groups(limit):
                st = q0_state
                while st["g0"] < limit:
                    g0 = st["g0"]
                    n = min(SLOT_GROUP, limit - g0)
                    pss = ps_sc.tile([128, n * TQ], F32, tag="sc")
                    for s in range(n):
                        kc, h = divmod(g0 + s, NH)
                        nc.tensor.matmul(
                            pss[:, TQ * s : TQ * (s + 1)],
                            kt[32 * h : 32 * (h + 1), 128 * kc : 128 * (kc + 1)],
                            qt[32 * h : 32 * (h + 1), 0:TQ],
                            start=True, stop=True,
                            tile_position=(32 * h, 0),
                            skip_group_check=True,
                        )
                    while st["next_kc"] < NKT and 4 * st["next_kc"] + 4 <= g0:
                        if st["av"] is None:
                            st["av"] = ps_av.tile([128, 2, TQ], F32, name="av")
                            st["emit_av"] = make_av_emitter(
                                st["av"], st["slot_ap"]
                            )
                        st["emit_av"](st["next_kc"])
                        st["next_kc"] += 1
                    att = attnp.tile([128, n * TQ], BF16, tag="at")
                    nc.scalar.activation(
                        att[:], pss[:], mybir.ActivationFunctionType.Exp
                    )
                    for s in range(n):
                        st["slot_ap"][g0 + s] = att[:, TQ * s : TQ * (s + 1)]
                    st["g0"] = g0 + n
                    st["gi"] += 1

            # ---- QKV projections, one pass over x^T, fused with q tile 0 -
            for t in range(NQT):
                tsl = slice(TQ * t, TQ * (t + 1))
                pst3 = ps_sc.tile([128, 3 * TQ], F32, tag="sc")
                psq = pst3[:, 0:TQ]
                psk = pst3[:, TQ : 2 * TQ]
                psv = pst3[:, 2 * TQ : 3 * TQ]
                for dm in range(NDM):
                    xt_t = xpool.tile([128, TQ], BF16)
                    nc.gpsimd.dma_start(xt_t, xT[128 * dm : 128 * (dm + 1), tsl])
                    xr = xt_t[:]
                    nc.tensor.matmul(
                        psq, wq_sb[:, dm, :], xr,
                        start=(dm == 0), stop=(dm == NDM - 1),
                        skip_group_check=True,
                    )
                    nc.tensor.matmul(
                        psk, wk_sb[:, dm, :], xr,
                        start=(dm == 0), stop=(dm == NDM - 1),
                        skip_group_check=True,
                    )
                    nc.tensor.matmul(
                        psv, wv_sb[:, dm, :], xr,
                        start=(dm == 0), stop=(dm == NDM - 1),
                        skip_group_check=True,
                    )
                nc.vector.tensor_scalar_add(qt[:, tsl], psq, bq_sb[:])
                nc.vector.tensor_scalar_add(kt[:, tsl], psk, bk_sb[:])
                nc.vector.tensor_scalar_add(vt_bf[:, tsl], psv, bv_sb[:])
                # this tile's 4 V chunks -> natural layout in the merged
                # [V|ones] stationary (single strided copy for all 4)
                tpt = ps_sc.tile([128, 4, 128], BF16, tag="sc")
                for c4 in range(4):
                    c = 4 * t + c4
                    nc.tensor.matmul(
                        tpt[:, c4, :], vt_bf[:, 128 * c : 128 * (c + 1)],
                        ident[:], is_transpose=True,
                        skip_group_check=True,
                    )
                nc.vector.tensor_copy(v_mg[:, 4 * t : 4 * t + 4, :, 0:32], tpt[:])

            # q tile 0 after the full projection: interleaving its score
            # groups INTO the proj loop measured slower -- the shared "sc"
            # rotation makes each proj tile's psum allocation wait on a
            # score group's exp, coupling the PE to ACT pacing.
            emit_q0_groups(N_SLOTS)

            pending = (
                0, q0_state["av"], q0_state["emit_av"], q0_state["next_kc"]
            )
            for q in range(1, NQT):
                qsl = slice(TQ * q, TQ * (q + 1))

                # AV/den accumulator: heads 0,1 in bank 0 (partitions 0-63 /
                # 64-127), heads 2,3 in bank 1. Within a head's 64
                # partitions: 0-31 = AV, 32-63 = denominator (x32 rows).
                # Allocated lazily at first use: it must be claimed AFTER
                # the previous tile's flush is emitted, or the pool rotation
                # mis-attributes the flush's reads to the new generation and
                # the next tile's accumulation races the normalize.
                av = None
                slot_ap = {}
                emit_av = None
                next_kc = 0
                g0 = 0
                gi = 0
                while g0 < N_SLOTS:
                    n = min(SLOT_GROUP, N_SLOTS - g0)
                    pss = ps_sc.tile([128, n * TQ], F32, tag="sc")
                    for s in range(n):
                        kc, h = divmod(g0 + s, NH)
                        nc.tensor.matmul(
                            pss[:, TQ * s : TQ * (s + 1)],
                            kt[32 * h : 32 * (h + 1), 128 * kc : 128 * (kc + 1)],
                            qt[32 * h : 32 * (h + 1), qsl],
                            start=True, stop=True,
                            tile_position=(32 * h, 0),
                            skip_group_check=True,
                        )
                    # Previous tile's AV tail + normalize + AllGather happen
                    # here, once this tile's first TWO score groups are in
                    # flight: ACT rolls from the old tile's exps straight
                    # into the new one's while the PE detours through the
                    # old tile's last AV chunk.
                    if gi == 1 and pending is not None:
                        flush_and_normalize(pending)
                        pending = None
                    # AV for every k chunk whose slots were exp'd at least
                    # one group ago (so the in-order PE never waits on the
                    # exp that was issued right before it); ACT chews the
                    # score group just produced in parallel.
                    while next_kc < NKT and 4 * next_kc + 4 <= g0:
                        if av is None:
                            av = ps_av.tile([128, 2, TQ], F32)
                            emit_av = make_av_emitter(av, slot_ap)
                        emit_av(next_kc)
                        next_kc += 1
                    att = attnp.tile([128, n * TQ], BF16, tag="at")
                    nc.scalar.activation(
                        att[:], pss[:], mybir.ActivationFunctionType.Exp
                    )
                    for s in range(n):
                        slot_ap[g0 + s] = att[:, TQ * s : TQ * (s + 1)]
                    g0 += n
                    gi += 1

                    # Epilogue at pipeline depth 2, mid-tile: its AllGather
                    # launched ~2.3 tiles ago, so even a badly skewed peer
                    # core can't stall the in-order PE here.
                    if q >= 2 and gi == 10:
                        emit_epilogue(q - 2)
                pending = (q, av, emit_av, next_kc)

            emit_epilogue(NQT - 2)
            flush_and_normalize(pending)
            emit_epilogue(NQT - 1)

            ps_av_ctx.__exit__(None, None, None)
            ps_sc_ctx.__exit__(None, None, None)

    nc.finalize()
    return nc


def _prepare_inputs(x, Wq, bq, Wk, bk, Wv, bv, Wo, bo):
    import ml_dtypes

    bf16 = ml_dtypes.bfloat16
    scale = 1.0 / math.sqrt(D_K)
    x = np.asarray(x, np.float32)
    in_maps = []
    for c in range(N_CORES):
        b, j = divmod(c, GROUP)
        rsl = slice(RLOC * j, RLOC * (j + 1))
        dsl = slice(512 * j, 512 * (j + 1))
        in_maps.append(
            {
                "xT": np.ascontiguousarray(x[b].T).astype(bf16),
                "wqT": np.ascontiguousarray((np.asarray(Wq)[rsl] * scale).T.astype(np.float32)).astype(bf16),
                "wkT": np.ascontiguousarray(np.asarray(Wk)[rsl].T.astype(np.float32)).astype(bf16),
                "wvT": np.ascontiguousarray(np.asarray(Wv)[rsl].T.astype(np.float32)).astype(bf16),
                "woTs": np.ascontiguousarray(np.asarray(Wo)[dsl].T.astype(np.float32)).astype(bf16),
                "bq": (np.asarray(bq)[rsl] * scale).astype(np.float32).reshape(RLOC, 1),
                "bk": np.asarray(bk)[rsl].astype(np.float32).reshape(RLOC, 1),
                "bv": np.asarray(bv)[rsl].astype(np.float32).reshape(RLOC, 1),
                "bo2": np.ascontiguousarray(
                    np.asarray(bo)[dsl].astype(np.float32).reshape(4, 128).T
                ),
            }
        )
    return in_maps


def kernel(x, Wq, bq, Wk, bk, Wv, bv, Wo, bo, mask=None):
    global LAST_RESULT
    from concourse.bass_utils import run_bass_kernel_spmd

    if "nc" not in _CACHE:
        _CACHE["nc"] = _build()
    nc = _CACHE["nc"]

    in_maps = _prepare_inputs(x, Wq, bq, Wk, bk, Wv, bv, Wo, bo)
    res = run_bass_kernel_spmd(
        nc, in_maps, core_ids=list(range(N_CORES)), trace=TRACE
    )
    LAST_RESULT = res
    out = np.empty((B, S, D_MODEL), np.float32)
    for c in range(N_CORES):
        b, j = divmod(c, GROUP)
        out[b, :, 512 * j : 512 * (j + 1)] = res.results[c]["outT"].T
    return out


# revision 25
# speedup vs baseline: 1.3123x; 1.0673x over previous
"""Trainium2 Bass kernel for LowDimProjectedAttention.

Model (reference):
  Q = x @ Wq.T + bq ; K,V likewise  (d_model=2048 -> r=512)
  16 heads of d_k=32, softmax(QK^T/sqrt(32)) @ V, then out_proj r->d_model.
  B=2, S=2048. mask is all-ones (verified by spec fill), dropout p=0.

Sharding (8 cores): core c handles batch b=c//4 and heads 4j..4j+4 where
j=c%4 (i.e. 128 of the 512 r-channels, column-parallel QKV). Attention is
fully local per core. A 4-way AllGather inside each batch group rebuilds
attn_out^T, after which each core computes a 512-wide slice of the output
d_model dimension (column-parallel out_proj, bias folded per-partition).

Layouts: all activations live transposed on-chip ([feature, token]); the
host pre-transposes x and the weights (cast to bf16) so no on-device
transpose of x is ever needed. Scores are computed as S^T[k,q] tiles.

Structure per q tile (the part that matters for speed):
  - scores come out in 3-slot PSUM groups, exp'd by ACT into bf16 attn
    tiles. ACT throughput (~0.83ns/elem) is the per-tile floor.
  - the AV matmul's stationary is [V_h | ones] (64 cols), so ONE matmul
    per (kc,h) yields both the weighted V sum and the softmax denominator
    broadcast over 32 partitions -- the separate ones-matmul of the naive
    version is folded away (halves attention-phase PE work).
  - AV matmuls are interleaved between score groups as soon as their attn
    slots are exp'd (one group of delay), so PE and ACT run concurrently
    all tile long.
  - tile q's AV tail + normalize + AllGather launch are emitted at the
    start of tile q+1 (ACT rolls straight across the boundary), and its
    out_proj is emitted mid-tile q+2: the AllGather then has ~2.3 tiles
    of latency budget, so the in-order PE queue doesn't stall on the
    collective even when a peer core runs tens of us late.

All matmuls run at 1 cycle/row (bf16 == float32r rate on TRN2); bf16 is
used for x/W/Q/K/V/attn and the collective to halve DMA + network bytes.
The softmax scale 1/sqrt(32) is folded into Wq/bq on the host.
"""

import math

import numpy as np

B = 2
S = 2048
D_MODEL = 2048
R = 512
N_HEADS = 16
D_K = 32
N_CORES = 8
GROUP = 4          # cores per batch group
RLOC = 128         # r-channels per core (4 heads x 32)
NH = 4             # heads per core
TQ = 512           # q tile size
NQT = S // TQ      # 4 q tiles
NKT = S // 128     # 16 k chunks
NDM = D_MODEL // 128  # 16 d_model chunks
SLOT_GROUP = 3     # score slots per ACT exp instruction (3 psum banks)
N_SLOTS = NKT * NH  # 64 score tiles per q tile: slot = 4*kc + h

_CACHE = {}
TRACE = False
LAST_RESULT = None


def _build():
    import concourse.mybir as mybir
    import concourse.tile as tile
    from concourse import bacc
    from concourse.masks import make_identity

    F32 = mybir.dt.float32
    BF16 = mybir.dt.bfloat16

    # Bacc (not plain Bass): its finalize() runs move_matmul_waits_to_
    # ldweights / generate_event_semaphores etc., without which walrus
    # rejects multi-wait instructions ("Too many sync wait commands").
    nc = bacc.Bacc("TRN2", target_bir_lowering=False, num_devices=N_CORES)

    xT = nc.dram_tensor("xT", [D_MODEL, S], BF16, kind="ExternalInput")
    wqT = nc.dram_tensor("wqT", [D_MODEL, RLOC], BF16, kind="ExternalInput")
    wkT = nc.dram_tensor("wkT", [D_MODEL, RLOC], BF16, kind="ExternalInput")
    wvT = nc.dram_tensor("wvT", [D_MODEL, RLOC], BF16, kind="ExternalInput")
    woTs = nc.dram_tensor("woTs", [R, 512], BF16, kind="ExternalInput")
    bq = nc.dram_tensor("bq", [RLOC, 1], F32, kind="ExternalInput")
    bk = nc.dram_tensor("bk", [RLOC, 1], F32, kind="ExternalInput")
    bv = nc.dram_tensor("bv", [RLOC, 1], F32, kind="ExternalInput")
    bo2 = nc.dram_tensor("bo2", [128, 4], F32, kind="ExternalInput")
    outT = nc.dram_tensor("outT", [512, S], F32, kind="ExternalOutput")

    # Per-q-tile collective bounce buffers (chunked AllGather overlaps the
    # epilogue with attention of later q tiles).
    cc_in = [
        nc.dram_tensor(f"cc_in{i}", [RLOC, TQ], BF16, kind="Internal")
        for i in range(NQT)
    ]
    # NOTE: Shared-output collectives need >4-core groups; Local output is
    # the supported path for 4-core batch groups (extra HBM copy, fine).
    cc_out = [
        nc.dram_tensor(f"cc_out{i}", [R, TQ], BF16, kind="Internal")
        for i in range(NQT)
    ]
    replica_groups = [[0, 1, 2, 3], [4, 5, 6, 7]]

    with tile.TileContext(nc) as tc:
        with (
            tc.tile_pool(name="const", bufs=1) as const,
            tc.tile_pool(name="wpool", bufs=1) as wpool,
            tc.tile_pool(name="xpool", bufs=4) as xpool,
            tc.tile_pool(name="qkv", bufs=1) as qkv,
            tc.tile_pool(name="attnp", bufs=10) as attnp,
            tc.tile_pool(name="denp", bufs=2) as denp,
            tc.tile_pool(name="otp", bufs=2) as otp,
            tc.tile_pool(name="agp", bufs=8) as agp,
            tc.tile_pool(name="outp", bufs=2) as outp,
        ):
            # ---- constants / weights -------------------------------------
            # chunked weight loads: one DMA per 128x128 chunk so each matmul
            # waits on a single DMA-queue semaphore (a single sprayed DMA
            # fans across queues and overflows the ISA wait slots).
            wq_sb = wpool.tile([128, NDM, RLOC], BF16)
            wk_sb = wpool.tile([128, NDM, RLOC], BF16)
            wv_sb = wpool.tile([128, NDM, RLOC], BF16)
            for dm in range(NDM):
                rs = slice(128 * dm, 128 * (dm + 1))
                nc.sync.dma_start(wq_sb[:, dm, :], wqT[rs, :])
                nc.sync.dma_start(wk_sb[:, dm, :], wkT[rs, :])
                nc.sync.dma_start(wv_sb[:, dm, :], wvT[rs, :])
            wo_sb = wpool.tile([128, 4, 4, 128], BF16)
            for rc in range(4):
                for dmt in range(4):
                    nc.sync.dma_start(
                        wo_sb[:, rc, dmt, :],
                        woTs[128 * rc : 128 * (rc + 1), 128 * dmt : 128 * (dmt + 1)],
                    )
            bq_sb = const.tile([RLOC, 1], F32)
            bk_sb = const.tile([RLOC, 1], F32)
            bv_sb = const.tile([RLOC, 1], F32)
            bo_sb = const.tile([128, 4], F32)
            nc.sync.dma_start(bq_sb, bq[:])
            nc.sync.dma_start(bk_sb, bk[:])
            nc.sync.dma_start(bv_sb, bv[:])
            nc.sync.dma_start(bo_sb, bo2[:])

            ident = const.tile([128, 128], BF16)
            make_identity(nc, ident[:])

            # Merged AV stationary: per (kc, h) a [128, 64] block whose
            # first 32 cols are V_h's chunk and last 32 cols are ones, so a
            # single matmul yields [AV_h ; denominator broadcast x32].
            v_mg = qkv.tile([128, NKT, NH, 64], BF16)
            nc.vector.memset(v_mg[:, :, :, 32:64], 1.0)

            qt = qkv.tile([RLOC, S], BF16)
            kt = qkv.tile([RLOC, S], BF16)
            vt_bf = qkv.tile([RLOC, S], BF16)

            # psum budget: 2x3-bank score groups (ping-pong under the exp
            # drain) + a 2-bank AV/den accumulator = exactly 8 banks.
            # QKV projection, V transposes and out_proj all borrow
            # score-pool banks (tag "sc") so both pools span the kernel.
            ps_sc_ctx = tc.tile_pool(name="ps_sc", bufs=2, space="PSUM")
            ps_av_ctx = tc.tile_pool(name="ps_av", bufs=1, space="PSUM")
            ps_sc = ps_sc_ctx.__enter__()
            ps_av = ps_av_ctx.__enter__()

            def emit_epilogue(qe):
                # out_proj for q tile qe (column-parallel over d_model).
                # Called one tile late so the AllGather is already done.
                qsl = slice(TQ * qe, TQ * (qe + 1))
                # gathers go on the Sync queue: on GpSimd they'd sit behind
                # the NEXT tile's collective_compute, which blocks that
                # queue until the AllGather completes -- a late peer then
                # stalls the in-order PE at the epilogue matmuls.
                ag_t = []
                for rc in range(GROUP):
                    t_ = agp.tile([128, TQ], BF16)
                    nc.sync.dma_start(
                        t_, cc_out[qe][128 * rc : 128 * (rc + 1), :]
                    )
                    ag_t.append(t_)
                for dmt in range(4):
                    pso2 = ps_sc.tile([128, TQ], F32, tag="sc")
                    for rc in range(GROUP):
                        nc.tensor.matmul(
                            pso2[:],
                            wo_sb[:, rc, dmt, :],
                            ag_t[rc][:],
                            start=(rc == 0), stop=(rc == GROUP - 1),
                            skip_group_check=True,
                        )
                    ob = outp.tile([128, TQ], F32)
                    nc.vector.tensor_scalar_add(
                        ob[:], pso2[:], bo_sb[:, dmt : dmt + 1]
                    )
                    nc.sync.dma_start(outT[128 * dmt : 128 * (dmt + 1), qsl], ob[:])

            # ---- attention: scores/exp/AV interleaved per q tile ---------
            def make_av_emitter(av, slot_ap):
                def emit_av(kc):
                    for h in range(NH):
                        p0 = 64 * (h % 2)
                        nc.tensor.matmul(
                            av[p0 : p0 + 64, h // 2, :],
                            v_mg[:, kc, h, :],
                            slot_ap[4 * kc + h],
                            start=(kc == 0), stop=(kc == NKT - 1),
                            tile_position=(0, p0),
                            skip_group_check=True,
                        )
                return emit_av

            def flush_and_normalize(pend):
                # Remaining AV chunks, then out = AV / denom, then ship.
                # (The denominator is already broadcast over each head's 32
                # rows by the ones columns of the merged stationary.)
                qf, av, emit_av, next_kc = pend
                for kc in range(next_kc, NKT):
                    emit_av(kc)
                # Stage the accumulator to SBUF with heads realigned to
                # partition band 32h (copies have a single SBUF operand, so
                # the verifier's same-start-partition rule for TensorTensor
                # doesn't bite). The psum banks are released for the next
                # tile's accumulation after these 8 copies (~4us), while the
                # slow exact reciprocal (DVE divide is ~8 cycles/elem; the
                # approx-fast custom op miscompiles under this runner) runs
                # off the critical path. Realignment also lets recip and the
                # AV*recip product each be ONE full-width instruction.
                av_al = denp.tile([128, TQ], F32, tag="aval")
                rbsrc = denp.tile([128, TQ], F32, tag="rbs")
                for h in range(NH):
                    p0 = 64 * (h % 2)
                    nc.vector.tensor_copy(
                        av_al[32 * h : 32 * (h + 1), :],
                        av[p0 : p0 + 32, h // 2, :],
                    )
                for h in range(NH):
                    p0 = 64 * (h % 2)
                    nc.vector.tensor_copy(
                        rbsrc[32 * h : 32 * (h + 1), :],
                        av[p0 + 32 : p0 + 64, h // 2, :],
                    )
                rb = denp.tile([128, TQ], F32, tag="rb")
                nc.vector.reciprocal(rb[:], rbsrc[:])
                ot = otp.tile([128, TQ], BF16)
                nc.vector.tensor_mul(ot[:], av_al[:], rb[:])
                nc.sync.dma_start(cc_in[qf][:], ot[:])
                # gather the 4 cores' head-slices of this q tile
                nc.gpsimd.collective_compute(
                    "AllGather",
                    mybir.AluOpType.bypass,
                    replica_groups=replica_groups,
                    ins=[cc_in[qf][:]],
                    outs=[cc_out[qf][:]],
                )

            # ---- q tile 0, emitted progressively inside the proj loop ----
            # After proj tile t lands (K/Q/V for tokens 512t..512t+512 and
            # the V chunks transposed into v_mg), score slots up to 16(t+1)
            # become legal: emitting them here keeps ACT busy from ~12us in
            # instead of idling through the whole projection phase.
            q0_state = {"g0": 0, "gi": 0, "next_kc": 0, "av": None,
                        "emit_av": None, "slot_ap": {}}

            def emit_q0_groups(limit):
                st = q0_state
                while st["g0"] < limit:
                    g0 = st["g0"]
                    n = min(SLOT_GROUP, limit - g0)
                    pss = ps_sc.tile([128, n * TQ], F32, tag="sc")
                    for s in range(n):
                        kc, h = divmod(g0 + s, NH)
                        nc.tensor.matmul(
                            pss[:, TQ * s : TQ * (s + 1)],
                            kt[32 * h : 32 * (h + 1), 128 * kc : 128 * (kc + 1)],
                            qt[32 * h : 32 * (h + 1), 0:TQ],
                            start=True, stop=True,
                            tile_position=(32 * h, 0),
                            skip_group_check=True,
                        )
                    while st["next_kc"] < NKT and 4 * st["next_kc"] + 4 <= g0:
                        if st["av"] is None:
                            st["av"] = ps_av.tile([128, 2, TQ], F32, name="av")
                            st["emit_av"] = make_av_emitter(
                                st["av"], st["slot_ap"]
                            )
                        st["emit_av"](st["next_kc"])
                        st["next_kc"] += 1
                    att = attnp.tile([128, n * TQ], BF16, tag="at")
                    nc.scalar.activation(
                        att[:], pss[:], mybir.ActivationFunctionType.Exp
                    )
                    for s in range(n):
                        st["slot_ap"][g0 + s] = att[:, TQ * s : TQ * (s + 1)]
                    st["g0"] = g0 + n
                    st["gi"] += 1

            # ---- QKV projections, one pass over x^T, fused with q tile 0 -
            for t in range(NQT):
                tsl = slice(TQ * t, TQ * (t + 1))
                pst3 = ps_sc.tile([128, 3 * TQ], F32, tag="sc")
                psq = pst3[:, 0:TQ]
                psk = pst3[:, TQ : 2 * TQ]
                psv = pst3[:, 2 * TQ : 3 * TQ]
                for dm in range(NDM):
                    xt_t = xpool.tile([128, TQ], BF16)
                    nc.gpsimd.dma_start(xt_t, xT[128 * dm : 128 * (dm + 1), tsl])
                    xr = xt_t[:]
                    nc.tensor.matmul(
                        psq, wq_sb[:, dm, :], xr,
                        start=(dm == 0), stop=(dm == NDM - 1),
                        skip_group_check=True,
                    )
                    nc.tensor.matmul(
                        psk, wk_sb[:, dm, :], xr,
                        start=(dm == 0), stop=(dm == NDM - 1),
                        skip_group_check=True,
                    )
                    nc.tensor.matmul(
                        psv, wv_sb[:, dm, :], xr,
                        start=(dm == 0), stop=(dm == NDM - 1),
                        skip_group_check=True,
                    )
                nc.vector.tensor_scalar_add(qt[:, tsl], psq, bq_sb[:])
                nc.vector.tensor_scalar_add(kt[:, tsl], psk, bk_sb[:])
                nc.vector.tensor_scalar_add(vt_bf[:, tsl], psv, bv_sb[:])
                # this tile's 4 V chunks -> natural layout in the merged
                # [V|ones] stationary (single strided copy for all 4)
                tpt = ps_sc.tile([128, 4, 128], BF16, tag="sc")
                for c4 in range(4):
                    c = 4 * t + c4
                    nc.tensor.matmul(
                        tpt[:, c4, :], vt_bf[:, 128 * c : 128 * (c + 1)],
                        ident[:], is_transpose=True,
                        skip_group_check=True,
                    )
                nc.vector.tensor_copy(v_mg[:, 4 * t : 4 * t + 4, :, 0:32], tpt[:])

            # q tile 0 after the full projection: interleaving its score
            # groups INTO the proj loop measured slower -- the shared "sc"
            # rotation makes each proj tile's psum allocation wait on a
            # score group's exp, coupling the PE to ACT pacing.
            emit_q0_groups(N_SLOTS)

            pending = (
                0, q0_state["av"], q0_state["emit_av"], q0_state["next_kc"]
            )
            for q in range(1, NQT):
                qsl = slice(TQ * q, TQ * (q + 1))

                # AV/den accumulator: heads 0,1 in bank 0 (partitions 0-63 /
                # 64-127), heads 2,3 in bank 1. Within a head's 64
                # partitions: 0-31 = AV, 32-63 = denominator (x32 rows).
                # Allocated lazily at first use: it must be claimed AFTER
                # the previous tile's flush is emitted, or the pool rotation
                # mis-attributes the flush's reads to the new generation and
                # the next tile's accumulation races the normalize.
                av = None
                slot_ap = {}
                emit_av = None
                next_kc = 0
                g0 = 0
                gi = 0
                while g0 < N_SLOTS:
                    n = min(SLOT_GROUP, N_SLOTS - g0)
                    pss = ps_sc.tile([128, n * TQ], F32, tag="sc")
                    for s in range(n):
                        kc, h = divmod(g0 + s, NH)
                        nc.tensor.matmul(
                            pss[:, TQ * s : TQ * (s + 1)],
                            kt[32 * h : 32 * (h + 1), 128 * kc : 128 * (kc + 1)],
                            qt[32 * h : 32 * (h + 1), qsl],
                            start=True, stop=True,
                            tile_position=(32 * h, 0),
                            skip_group_check=True,
                        )
                    # Previous tile's AV tail + normalize + AllGather happen
                    # here, after this tile's first score group: ACT rolls
                    # straight from the old tile's exps into the new one's.
                    # (Flushing one group later measured ~35us slower.)
                    if gi == 0 and pending is not None:
                        flush_and_normalize(pending)
                        pending = None
                    # AV for every k chunk whose slots were exp'd at least
                    # one group ago (so the in-order PE never waits on the
                    # exp that was issued right before it); ACT chews the
                    # score group just produced in parallel.
                    while next_kc < NKT and 4 * next_kc + 4 <= g0:
                        if av is None:
                            av = ps_av.tile([128, 2, TQ], F32)
                            emit_av = make_av_emitter(av, slot_ap)
                        emit_av(next_kc)
                        next_kc += 1
                    att = attnp.tile([128, n * TQ], BF16, tag="at")
                    nc.scalar.activation(
                        att[:], pss[:], mybir.ActivationFunctionType.Exp
                    )
                    for s in range(n):
                        slot_ap[g0 + s] = att[:, TQ * s : TQ * (s + 1)]
                    g0 += n
                    gi += 1

                    # Epilogue at pipeline depth 2, mid-tile: its AllGather
                    # launched ~2.3 tiles ago, so even a badly skewed peer
                    # core can't stall the in-order PE here.
                    if q >= 2 and gi == 10:
                        emit_epilogue(q - 2)
                pending = (q, av, emit_av, next_kc)

            emit_epilogue(NQT - 2)
            flush_and_normalize(pending)
            emit_epilogue(NQT - 1)

            ps_av_ctx.__exit__(None, None, None)
            ps_sc_ctx.__exit__(None, None, None)

    nc.finalize()
    return nc


def _prepare_inputs(x, Wq, bq, Wk, bk, Wv, bv, Wo, bo):
    import ml_dtypes

    bf16 = ml_dtypes.bfloat16
    scale = 1.0 / math.sqrt(D_K)
    x = np.asarray(x, np.float32)
    in_maps = []
    for c in range(N_CORES):
        b, j = divmod(c, GROUP)
        rsl = slice(RLOC * j, RLOC * (j + 1))
        dsl = slice(512 * j, 512 * (j + 1))
        in_maps.append(
            {
                "xT": np.ascontiguousarray(x[b].T).astype(bf16),
                "wqT": np.ascontiguousarray((np.asarray(Wq)[rsl] * scale).T.astype(np.float32)).astype(bf16),
                "wkT": np.ascontiguousarray(np.asarray(Wk)[rsl].T.astype(np.float32)).astype(bf16),
                "wvT": np.ascontiguousarray(np.asarray(Wv)[rsl].T.astype(np.float32)).astype(bf16),
                "woTs": np.ascontiguousarray(np.asarray(Wo)[dsl].T.astype(np.float32)).astype(bf16),
                "bq": (np.asarray(bq)[rsl] * scale).astype(np.float32).reshape(RLOC, 1),
                "bk": np.asarray(bk)[rsl].astype(np.float32).reshape(RLOC, 1),
                "bv": np.asarray(bv)[rsl].astype(np.float32).reshape(RLOC, 1),
                "bo2": np.ascontiguousarray(
                    np.asarray(bo)[dsl].astype(np.float32).reshape(4, 128).T
                ),
            }
        )
    return in_maps


def kernel(x, Wq, bq, Wk, bk, Wv, bv, Wo, bo, mask=None):
    global LAST_RESULT
    from concourse.bass_utils import run_bass_kernel_spmd

    if "nc" not in _CACHE:
        _CACHE["nc"] = _build()
    nc = _CACHE["nc"]

    in_maps = _prepare_inputs(x, Wq, bq, Wk, bk, Wv, bv, Wo, bo)
    res = run_bass_kernel_spmd(
        nc, in_maps, core_ids=list(range(N_CORES)), trace=TRACE
    )
    LAST_RESULT = res
    out = np.empty((B, S, D_MODEL), np.float32)
    for c in range(N_CORES):
        b, j = divmod(c, GROUP)
        out[b, :, 512 * j : 512 * (j + 1)] = res.results[c]["outT"].T
    return out


# revision 28
# speedup vs baseline: 1.3795x; 1.0512x over previous
"""Trainium2 Bass kernel for LowDimProjectedAttention.

Model (reference):
  Q = x @ Wq.T + bq ; K,V likewise  (d_model=2048 -> r=512)
  16 heads of d_k=32, softmax(QK^T/sqrt(32)) @ V, then out_proj r->d_model.
  B=2, S=2048. mask is all-ones (verified by spec fill), dropout p=0.

Sharding (8 cores): core c handles batch b=c//4 and heads 4j..4j+4 where
j=c%4 (i.e. 128 of the 512 r-channels, column-parallel QKV). Attention is
fully local per core. A 4-way AllGather inside each batch group rebuilds
attn_out^T, after which each core computes a 512-wide slice of the output
d_model dimension (column-parallel out_proj, bias folded per-partition).

Layouts: all activations live transposed on-chip ([feature, token]); the
host pre-transposes x and the weights (cast to bf16) so no on-device
transpose of x is ever needed. Scores are computed as S^T[k,q] tiles.

Structure per q tile (the part that matters for speed):
  - scores come out in 3-slot PSUM groups, exp'd by ACT into bf16 attn
    tiles. ACT throughput (~0.83ns/elem) is the per-tile floor.
  - the AV matmul's stationary is [V_h | ones] (64 cols), so ONE matmul
    per (kc,h) yields both the weighted V sum and the softmax denominator
    broadcast over 32 partitions -- the separate ones-matmul of the naive
    version is folded away (halves attention-phase PE work).
  - AV matmuls are interleaved between score groups as soon as their attn
    slots are exp'd (one group of delay), so PE and ACT run concurrently
    all tile long.
  - tile q's AV tail + normalize + AllGather launch are emitted at the
    start of tile q+1 (ACT rolls straight across the boundary), and its
    out_proj is emitted mid-tile q+2: the AllGather then has ~2.3 tiles
    of latency budget, so the in-order PE queue doesn't stall on the
    collective even when a peer core runs tens of us late.

All matmuls run at 1 cycle/row (bf16 == float32r rate on TRN2); bf16 is
used for x/W/Q/K/V/attn and the collective to halve DMA + network bytes.
The softmax scale 1/sqrt(32) is folded into Wq/bq on the host.
"""

import math

import numpy as np

B = 2
S = 2048
D_MODEL = 2048
R = 512
N_HEADS = 16
D_K = 32
N_CORES = 8
GROUP = 4          # cores per batch group
RLOC = 128         # r-channels per core (4 heads x 32)
NH = 4             # heads per core
TQ = 512           # q tile size
NQT = S // TQ      # 4 q tiles
NKT = S // 128     # 16 k chunks
NDM = D_MODEL // 128  # 16 d_model chunks
SLOT_GROUP = 3     # score slots per ACT exp instruction (3 psum banks)
N_SLOTS = NKT * NH  # 64 score tiles per q tile: slot = 4*kc + h

_CACHE = {}
TRACE = False
LAST_RESULT = None


def _build():
    import concourse.mybir as mybir
    import concourse.tile as tile
    from concourse import bacc
    from concourse.masks import make_identity

    F32 = mybir.dt.float32
    BF16 = mybir.dt.bfloat16

    # Bacc (not plain Bass): its finalize() runs move_matmul_waits_to_
    # ldweights / generate_event_semaphores etc., without which walrus
    # rejects multi-wait instructions ("Too many sync wait commands").
    nc = bacc.Bacc("TRN2", target_bir_lowering=False, num_devices=N_CORES)

    xT = nc.dram_tensor("xT", [D_MODEL, S], BF16, kind="ExternalInput")
    wqT = nc.dram_tensor("wqT", [D_MODEL, RLOC], BF16, kind="ExternalInput")
    wkT = nc.dram_tensor("wkT", [D_MODEL, RLOC], BF16, kind="ExternalInput")
    wvT = nc.dram_tensor("wvT", [D_MODEL, RLOC], BF16, kind="ExternalInput")
    woTs = nc.dram_tensor("woTs", [R, 512], BF16, kind="ExternalInput")
    bq = nc.dram_tensor("bq", [RLOC, 1], F32, kind="ExternalInput")
    bk = nc.dram_tensor("bk", [RLOC, 1], F32, kind="ExternalInput")
    bv = nc.dram_tensor("bv", [RLOC, 1], F32, kind="ExternalInput")
    bo2 = nc.dram_tensor("bo2", [128, 4], F32, kind="ExternalInput")
    outT = nc.dram_tensor("outT", [512, S], F32, kind="ExternalOutput")

    # Per-q-tile collective bounce buffers (chunked AllGather overlaps the
    # epilogue with attention of later q tiles).
    cc_in = [
        nc.dram_tensor(f"cc_in{i}", [RLOC, TQ], BF16, kind="Internal")
        for i in range(NQT)
    ]
    # NOTE: Shared-output collectives need >4-core groups; Local output is
    # the supported path for 4-core batch groups (extra HBM copy, fine).
    cc_out = [
        nc.dram_tensor(f"cc_out{i}", [R, TQ], BF16, kind="Internal")
        for i in range(NQT)
    ]
    replica_groups = [[0, 1, 2, 3], [4, 5, 6, 7]]

    with tile.TileContext(nc) as tc:
        with (
            tc.tile_pool(name="const", bufs=1) as const,
            tc.tile_pool(name="wpool", bufs=1) as wpool,
            tc.tile_pool(name="xpool", bufs=4) as xpool,
            tc.tile_pool(name="qkv", bufs=1) as qkv,
            tc.tile_pool(name="attnp", bufs=10) as attnp,
            tc.tile_pool(name="denp", bufs=2) as denp,
            tc.tile_pool(name="otp", bufs=2) as otp,
            tc.tile_pool(name="agp", bufs=8) as agp,
            tc.tile_pool(name="outp", bufs=2) as outp,
        ):
            # ---- constants / weights -------------------------------------
            # chunked weight loads: one DMA per 128x128 chunk so each matmul
            # waits on a single DMA-queue semaphore (a single sprayed DMA
            # fans across queues and overflows the ISA wait slots).
            wq_sb = wpool.tile([128, NDM, RLOC], BF16)
            wk_sb = wpool.tile([128, NDM, RLOC], BF16)
            wv_sb = wpool.tile([128, NDM, RLOC], BF16)
            for dm in range(NDM):
                rs = slice(128 * dm, 128 * (dm + 1))
                nc.sync.dma_start(wq_sb[:, dm, :], wqT[rs, :])
                nc.sync.dma_start(wk_sb[:, dm, :], wkT[rs, :])
                nc.sync.dma_start(wv_sb[:, dm, :], wvT[rs, :])
            wo_sb = wpool.tile([128, 4, 4, 128], BF16)
            for rc in range(4):
                for dmt in range(4):
                    nc.sync.dma_start(
                        wo_sb[:, rc, dmt, :],
                        woTs[128 * rc : 128 * (rc + 1), 128 * dmt : 128 * (dmt + 1)],
                    )
            bq_sb = const.tile([RLOC, 1], F32)
            bk_sb = const.tile([RLOC, 1], F32)
            bv_sb = const.tile([RLOC, 1], F32)
            bo_sb = const.tile([128, 4], F32)
            nc.sync.dma_start(bq_sb, bq[:])
            nc.sync.dma_start(bk_sb, bk[:])
            nc.sync.dma_start(bv_sb, bv[:])
            nc.sync.dma_start(bo_sb, bo2[:])

            ident = const.tile([128, 128], BF16)
            make_identity(nc, ident[:])

            # Merged AV stationary: per (kc, h) a [128, 64] block whose
            # first 32 cols are V_h's chunk and last 32 cols are ones, so a
            # single matmul yields [AV_h ; denominator broadcast x32].
            v_mg = qkv.tile([128, NKT, NH, 64], BF16)
            nc.vector.memset(v_mg[:, :, :, 32:64], 1.0)

            qt = qkv.tile([RLOC, S], BF16)
            kt = qkv.tile([RLOC, S], BF16)
            vt_bf = qkv.tile([RLOC, S], BF16)

            # psum budget: 2x3-bank score groups (ping-pong under the exp
            # drain) + a 2-bank AV/den accumulator = exactly 8 banks.
            # QKV projection, V transposes and out_proj all borrow
            # score-pool banks (tag "sc") so both pools span the kernel.
            ps_sc_ctx = tc.tile_pool(name="ps_sc", bufs=2, space="PSUM")
            ps_av_ctx = tc.tile_pool(name="ps_av", bufs=1, space="PSUM")
            ps_sc = ps_sc_ctx.__enter__()
            ps_av = ps_av_ctx.__enter__()

            def emit_epilogue(qe):
                # out_proj for q tile qe (column-parallel over d_model).
                # Called one tile late so the AllGather is already done.
                qsl = slice(TQ * qe, TQ * (qe + 1))
                # gathers go on the Sync queue: on GpSimd they'd sit behind
                # the NEXT tile's collective_compute, which blocks that
                # queue until the AllGather completes -- a late peer then
                # stalls the in-order PE at the epilogue matmuls.
                ag_t = []
                for rc in range(GROUP):
                    t_ = agp.tile([128, TQ], BF16)
                    nc.sync.dma_start(
                        t_, cc_out[qe][128 * rc : 128 * (rc + 1), :]
                    )
                    ag_t.append(t_)
                for dmt in range(4):
                    pso2 = ps_sc.tile([128, TQ], F32, tag="sc")
                    for rc in range(GROUP):
                        nc.tensor.matmul(
                            pso2[:],
                            wo_sb[:, rc, dmt, :],
                            ag_t[rc][:],
                            start=(rc == 0), stop=(rc == GROUP - 1),
                            skip_group_check=True,
                        )
                    ob = outp.tile([128, TQ], F32)
                    nc.vector.tensor_scalar_add(
                        ob[:], pso2[:], bo_sb[:, dmt : dmt + 1]
                    )
                    nc.sync.dma_start(outT[128 * dmt : 128 * (dmt + 1), qsl], ob[:])

            # ---- attention: scores/exp/AV interleaved per q tile ---------
            def make_av_emitter(av, slot_ap):
                def emit_av(kc):
                    for h in range(NH):
                        p0 = 64 * (h % 2)
                        nc.tensor.matmul(
                            av[p0 : p0 + 64, h // 2, :],
                            v_mg[:, kc, h, :],
                            slot_ap[4 * kc + h],
                            start=(kc == 0), stop=(kc == NKT - 1),
                            tile_position=(0, p0),
                            skip_group_check=True,
                        )
                return emit_av

            def flush_and_normalize(pend):
                # Remaining AV chunks, then out = AV / denom, then ship.
                # (The denominator is already broadcast over each head's 32
                # rows by the ones columns of the merged stationary.)
                qf, av, emit_av, next_kc = pend
                for kc in range(next_kc, NKT):
                    emit_av(kc)
                # Stage the accumulator to SBUF with heads realigned to
                # partition band 32h (copies have a single SBUF operand, so
                # the verifier's same-start-partition rule for TensorTensor
                # doesn't bite). The psum banks are released for the next
                # tile's accumulation after these 8 copies (~4us), while the
                # slow exact reciprocal (DVE divide is ~8 cycles/elem; the
                # approx-fast custom op miscompiles under this runner) runs
                # off the critical path. Realignment also lets recip and the
                # AV*recip product each be ONE full-width instruction.
                av_al = denp.tile([128, TQ], F32, tag="aval")
                rbsrc = denp.tile([128, TQ], F32, tag="rbs")
                for h in range(NH):
                    p0 = 64 * (h % 2)
                    nc.vector.tensor_copy(
                        av_al[32 * h : 32 * (h + 1), :],
                        av[p0 : p0 + 32, h // 2, :],
                    )
                for h in range(NH):
                    p0 = 64 * (h % 2)
                    nc.vector.tensor_copy(
                        rbsrc[32 * h : 32 * (h + 1), :],
                        av[p0 + 32 : p0 + 64, h // 2, :],
                    )
                rb = denp.tile([128, TQ], F32, tag="rb")
                nc.vector.reciprocal(rb[:], rbsrc[:])
                ot = otp.tile([128, TQ], BF16)
                nc.vector.tensor_mul(ot[:], av_al[:], rb[:])
                nc.sync.dma_start(cc_in[qf][:], ot[:])
                # gather the 4 cores' head-slices of this q tile
                nc.gpsimd.collective_compute(
                    "AllGather",
                    mybir.AluOpType.bypass,
                    replica_groups=replica_groups,
                    ins=[cc_in[qf][:]],
                    outs=[cc_out[qf][:]],
                )

            # ---- q tile 0, emitted progressively inside the proj loop ----
            # After proj tile t lands (K/Q/V for tokens 512t..512t+512 and
            # the V chunks transposed into v_mg), score slots up to 16(t+1)
            # become legal: emitting them here keeps ACT busy from ~12us in
            # instead of idling through the whole projection phase.
            q0_state = {"g0": 0, "gi": 0, "next_kc": 0, "av": None,
                        "emit_av": None, "slot_ap": {}}

            def emit_q0_groups(limit):
                st = q0_state
                while st["g0"] < limit:
                    g0 = st["g0"]
                    n = min(SLOT_GROUP, limit - g0)
                    pss = ps_sc.tile([128, n * TQ], F32, tag="sc")
                    for s in range(n):
                        kc, h = divmod(g0 + s, NH)
                        nc.tensor.matmul(
                            pss[:, TQ * s : TQ * (s + 1)],
                            kt[32 * h : 32 * (h + 1), 128 * kc : 128 * (kc + 1)],
                            qt[32 * h : 32 * (h + 1), 0:TQ],
                            start=True, stop=True,
                            tile_position=(32 * h, 0),
                            skip_group_check=True,
                        )
                    while st["next_kc"] < NKT and 4 * st["next_kc"] + 4 <= g0:
                        if st["av"] is None:
                            st["av"] = ps_av.tile([128, 2, TQ], F32, name="av")
                            st["emit_av"] = make_av_emitter(
                                st["av"], st["slot_ap"]
                            )
                        st["emit_av"](st["next_kc"])
                        st["next_kc"] += 1
                    att = attnp.tile([128, n * TQ], BF16, tag="at")
                    nc.scalar.activation(
                        att[:], pss[:], mybir.ActivationFunctionType.Exp
                    )
                    for s in range(n):
                        st["slot_ap"][g0 + s] = att[:, TQ * s : TQ * (s + 1)]
                    st["g0"] = g0 + n
                    st["gi"] += 1

            # ---- QKV projections, one pass over x^T, fused with q tile 0 -
            for t in range(NQT):
                tsl = slice(TQ * t, TQ * (t + 1))
                pst3 = ps_sc.tile([128, 3 * TQ], F32, tag="sc")
                psq = pst3[:, 0:TQ]
                psk = pst3[:, TQ : 2 * TQ]
                psv = pst3[:, 2 * TQ : 3 * TQ]
                for dm in range(NDM):
                    xt_t = xpool.tile([128, TQ], BF16)
                    # alternate DMA queues: one queue issues DIRECT2D at
                    # only ~630ns/instr, which barely keeps ahead of the
                    # PE's 3 matmuls per chunk; two queues double the feed.
                    # ACT's queue is free here -- exp work only starts
                    # after the projection phase.
                    dq = nc.gpsimd if dm % 2 == 0 else nc.scalar
                    dq.dma_start(xt_t, xT[128 * dm : 128 * (dm + 1), tsl])
                    xr = xt_t[:]
                    nc.tensor.matmul(
                        psq, wq_sb[:, dm, :], xr,
                        start=(dm == 0), stop=(dm == NDM - 1),
                        skip_group_check=True,
                    )
                    nc.tensor.matmul(
                        psk, wk_sb[:, dm, :], xr,
                        start=(dm == 0), stop=(dm == NDM - 1),
                        skip_group_check=True,
                    )
                    nc.tensor.matmul(
                        psv, wv_sb[:, dm, :], xr,
                        start=(dm == 0), stop=(dm == NDM - 1),
                        skip_group_check=True,
                    )
                nc.vector.tensor_scalar_add(qt[:, tsl], psq, bq_sb[:])
                nc.vector.tensor_scalar_add(kt[:, tsl], psk, bk_sb[:])
                nc.vector.tensor_scalar_add(vt_bf[:, tsl], psv, bv_sb[:])
                # this tile's 4 V chunks -> natural layout in the merged
                # [V|ones] stationary (single strided copy for all 4)
                tpt = ps_sc.tile([128, 4, 128], BF16, tag="sc")
                for c4 in range(4):
                    c = 4 * t + c4
                    nc.tensor.matmul(
                        tpt[:, c4, :], vt_bf[:, 128 * c : 128 * (c + 1)],
                        ident[:], is_transpose=True,
                        skip_group_check=True,
                    )
                nc.vector.tensor_copy(v_mg[:, 4 * t : 4 * t + 4, :, 0:32], tpt[:])

            # q tile 0 after the full projection: interleaving its score
            # groups INTO the proj loop measured slower -- the shared "sc"
            # rotation makes each proj tile's psum allocation wait on a
            # score group's exp, coupling the PE to ACT pacing.
            emit_q0_groups(N_SLOTS)

            pending = (
                0, q0_state["av"], q0_state["emit_av"], q0_state["next_kc"]
            )
            for q in range(1, NQT):
                qsl = slice(TQ * q, TQ * (q + 1))

                # AV/den accumulator: heads 0,1 in bank 0 (partitions 0-63 /
                # 64-127), heads 2,3 in bank 1. Within a head's 64
                # partitions: 0-31 = AV, 32-63 = denominator (x32 rows).
                # Allocated lazily at first use: it must be claimed AFTER
                # the previous tile's flush is emitted, or the pool rotation
                # mis-attributes the flush's reads to the new generation and
                # the next tile's accumulation races the normalize.
                av = None
                slot_ap = {}
                emit_av = None
                next_kc = 0
                g0 = 0
                gi = 0
                while g0 < N_SLOTS:
                    n = min(SLOT_GROUP, N_SLOTS - g0)
                    pss = ps_sc.tile([128, n * TQ], F32, tag="sc")
                    for s in range(n):
                        kc, h = divmod(g0 + s, NH)
                        nc.tensor.matmul(
                            pss[:, TQ * s : TQ * (s + 1)],
                            kt[32 * h : 32 * (h + 1), 128 * kc : 128 * (kc + 1)],
                            qt[32 * h : 32 * (h + 1), qsl],
                            start=True, stop=True,
                            tile_position=(32 * h, 0),
                            skip_group_check=True,
                        )
                    # Previous tile's AV tail + normalize + AllGather happen
                    # here, after this tile's first score group: ACT rolls
                    # straight from the old tile's exps into the new one's.
                    # (Flushing one group later measured ~35us slower.)
                    if gi == 0 and pending is not None:
                        flush_and_normalize(pending)
                        pending = None
                    # AV for every k chunk whose slots were exp'd at least
                    # one group ago (so the in-order PE never waits on the
                    # exp that was issued right before it); ACT chews the
                    # score group just produced in parallel.
                    while next_kc < NKT and 4 * next_kc + 4 <= g0:
                        if av is None:
                            av = ps_av.tile([128, 2, TQ], F32)
                            emit_av = make_av_emitter(av, slot_ap)
                        emit_av(next_kc)
                        next_kc += 1
                    att = attnp.tile([128, n * TQ], BF16, tag="at")
                    nc.scalar.activation(
                        att[:], pss[:], mybir.ActivationFunctionType.Exp
                    )
                    for s in range(n):
                        slot_ap[g0 + s] = att[:, TQ * s : TQ * (s + 1)]
                    g0 += n
                    gi += 1

                    # Epilogue at pipeline depth 2, mid-tile: its AllGather
                    # launched ~2.3 tiles ago, so even a badly skewed peer
                    # core can't stall the in-order PE here.
                    if q >= 2 and gi == 10:
                        emit_epilogue(q - 2)
                pending = (q, av, emit_av, next_kc)

            # Flush (and launch the final AllGather) FIRST: epilogue(2)'s
            # PE work then fills part of the last collective's latency
            # window instead of extending the exposed tail.
            flush_and_normalize(pending)
            emit_epilogue(NQT - 2)
            emit_epilogue(NQT - 1)

            ps_av_ctx.__exit__(None, None, None)
            ps_sc_ctx.__exit__(None, None, None)

    nc.finalize()
    return nc


def _prepare_inputs(x, Wq, bq, Wk, bk, Wv, bv, Wo, bo):
    import ml_dtypes

    bf16 = ml_dtypes.bfloat16
    scale = 1.0 / math.sqrt(D_K)
    x = np.asarray(x, np.float32)
    in_maps = []
    for c in range(N_CORES):
        b, j = divmod(c, GROUP)
        rsl = slice(RLOC * j, RLOC * (j + 1))
        dsl = slice(512 * j, 512 * (j + 1))
        in_maps.append(
            {
                "xT": np.ascontiguousarray(x[b].T).astype(bf16),
                "wqT": np.ascontiguousarray((np.asarray(Wq)[rsl] * scale).T.astype(np.float32)).astype(bf16),
                "wkT": np.ascontiguousarray(np.asarray(Wk)[rsl].T.astype(np.float32)).astype(bf16),
                "wvT": np.ascontiguousarray(np.asarray(Wv)[rsl].T.astype(np.float32)).astype(bf16),
                "woTs": np.ascontiguousarray(np.asarray(Wo)[dsl].T.astype(np.float32)).astype(bf16),
                "bq": (np.asarray(bq)[rsl] * scale).astype(np.float32).reshape(RLOC, 1),
                "bk": np.asarray(bk)[rsl].astype(np.float32).reshape(RLOC, 1),
                "bv": np.asarray(bv)[rsl].astype(np.float32).reshape(RLOC, 1),
                "bo2": np.ascontiguousarray(
                    np.asarray(bo)[dsl].astype(np.float32).reshape(4, 128).T
                ),
            }
        )
    return in_maps


def kernel(x, Wq, bq, Wk, bk, Wv, bv, Wo, bo, mask=None):
    global LAST_RESULT
    from concourse.bass_utils import run_bass_kernel_spmd

    if "nc" not in _CACHE:
        _CACHE["nc"] = _build()
    nc = _CACHE["nc"]

    in_maps = _prepare_inputs(x, Wq, bq, Wk, bk, Wv, bv, Wo, bo)
    res = run_bass_kernel_spmd(
        nc, in_maps, core_ids=list(range(N_CORES)), trace=TRACE
    )
    LAST_RESULT = res
    out = np.empty((B, S, D_MODEL), np.float32)
    for c in range(N_CORES):
        b, j = divmod(c, GROUP)
        out[b, :, 512 * j : 512 * (j + 1)] = res.results[c]["outT"].T
    return out
